# revision 45
# baseline (speedup 1.0000x reference)
"""Trainium2 Bass kernel for a 1-layer transformer encoder.

Sharding: data-parallel over batch -- each of the 8 cores gets 4 full
sequences (2048 tokens) and all weights; no collectives.

Structure (547us vs the 618us reproduced / 683us stated baseline):
- Per head-pair m: project qT/kT for pair m+1 software-pipelined between
  this pair's score halves and attnV, so the PE streams projections while
  softmax exp drains on ACT/DVE.
- Score matmuls for the two heads of a pair emitted adjacently at PE tile
  positions (0,0)/(64,0); the row-groups execute concurrently (~2x).
- 50% of exp tiles via a 1-op DVE bit-trick (Schraudolph in bf16 space:
  i16 = s*log2e*128 + 16249.6, bitcast to bf16), rest accurate on ACT.
- Softmax normalize: den row from the ones-column of V, ACT copy to SBUF,
  DVE reciprocal_approx_fast, gpsimd partition_broadcast, one fused
  psum*bcast multiply into oT (no DRAM bounce).
- LN: bn_stats/bn_aggr on DVE, sqrt on ACT, normalize via ACT Identity
  with per-partition scale/bias; gain/bias elided when inputs are the
  literal ones/zeros from setup_inputs (general path kept as fallback);
  b2 bias matmuls skipped when b2 == 0.
- out1 kept in SBUF as bf16 (no DRAM spill); o1 transposes + copies
  emitted inside the Wo loop so FFN starts without queue stalls; w1
  loaded column-major; transpose copies batched 4-per-instruction;
  PSUM: 2x[128,1024] + 2x[128,512] + 2x[65,512] tags = 8 banks.

Known hardware quirks worked around: reciprocal_approx_fast misreads
PSUM at partition base 64; partition bases must be 32-aligned; DVE ops
may read only one PSUM operand; ACT table reloads cost 1.28us.
"""

import os
import sys

import numpy as np

for _p in ("/opt/trn_rl_repo", "/root/.axon_site/_ro/trn_rl_repo"):
    if os.path.isdir(_p) and _p not in sys.path:
        sys.path.insert(0, _p)

os.environ.setdefault("MYCRO_LOCAL_CACHE", "1")

import concourse.bacc as bacc
import concourse.tile as tile
from concourse import mybir
from concourse.bass_utils import run_bass_kernel_spmd
from concourse.masks import make_identity

F32 = mybir.dt.float32
BF16 = mybir.dt.bfloat16
I16 = mybir.dt.int16
AF = mybir.ActivationFunctionType
OP = mybir.AluOpType

# model dims
D, NHEAD, HD, FF, SEQ, P = 768, 12, 64, 3072, 512, 128
ND = D // P    # 6 feature chunks
NF = FF // P   # 24 hidden chunks
EPS = 1e-5
N_CORES = 8
B_TOTAL = 32
DA = D + 1     # aug width (row-sum col for LN mean)

# Schraudolph fast-exp in bf16 bit space: bits_i16 = s*log2e*128 + C2
FE_C1 = float(1.4426950408889634 * 128.0)
FE_C2 = 16249.6

FAST_TILES = (1, 3)   # which of the 4 (half,head) exp tiles use DVE fastexp


def _ln_norm(nc, pool, ps, tag, eps_t, y_out):
    """y_out = layernorm(ps[:, :D]).

    Stats on DVE (bn_stats), sqrt on ACT, normalize on ACT (Identity with
    per-partition scale/bias) -- keeps both engines lightly loaded."""
    stats = pool.tile([P, 3, 6], F32, tag=f"bs{tag}", bufs=2, name=f"bs{tag}")
    for sg in range(3):
        nc.vector.bn_stats(out=stats[:, sg, :], in_=ps[:, sg * 256:(sg + 1) * 256])
    mv = pool.tile([P, 2], F32, tag=f"mv{tag}", bufs=2, name=f"mv{tag}")
    nc.vector.bn_aggr(out=mv, in_=stats)
    sd = pool.tile([P, 1], F32, tag=f"sd{tag}", bufs=2, name=f"sd{tag}")
    nc.scalar.activation(out=sd, in_=mv[:, 1:2], func=AF.Sqrt, bias=eps_t,
                         scale=1.0)
    rstd = pool.tile([P, 1], F32, tag=f"rstd{tag}", bufs=2, name=f"rstd{tag}")
    nc.vector.reciprocal_approx_fast(out=rstd, in_=sd)
    nm = pool.tile([P, 1], F32, tag=f"nm{tag}", bufs=2, name=f"nm{tag}")
    nc.vector.tensor_scalar(out=nm, in0=mv[:, 0:1], scalar1=rstd, scalar2=-1.0,
                            op0=OP.mult, op1=OP.mult)
    nc.scalar.activation(out=y_out, in_=ps[:, 0:D], func=AF.Identity,
                         bias=nm, scale=rstd)


def emit(nc, tc, io, bpc, ln_trivial, bias_zero):
    T = bpc * SEQ
    NT = T // P          # 16 token chunks
    NB = bpc             # 4 batches
    VW = NHEAD * (HD + 1)

    consts = tc.alloc_tile_pool(name="consts", bufs=1)
    psp = tc.alloc_tile_pool(name="psp", bufs=1, space="PSUM")

    # ---- constants ----
    ident = consts.tile([P, P], BF16)
    make_identity(nc, ident)
    ones1 = consts.tile([1, P], BF16)
    nc.vector.memset(ones1, 1.0)
    eps_t = consts.tile([P, 1], F32)
    nc.vector.memset(eps_t, EPS)
    b2r = consts.tile([1, DA], BF16)
    b1t = consts.tile([P, NF], F32)
    lnw = {}
    if not ln_trivial:
        for nm in ("ln1w", "ln1b", "ln2w", "ln2b"):
            t = consts.tile([P, D], F32, tag=nm, name=nm)
            nc.gpsimd.dma_start(t, io[nm].broadcast_to([P, D]))
            lnw[nm] = t

    # ---- right stack ----
    qkvp = tc.alloc_tile_pool(name="qkvp", bufs=1, side="right")
    qT = [qkvp.tile([P, T], BF16, tag=f"qT{j}", name=f"qT{j}") for j in range(ND)]
    kT = [qkvp.tile([P, T], BF16, tag=f"kT{j}", name=f"kT{j}") for j in range(ND)]
    vN = [qkvp.tile([P, VW], BF16, tag=f"v{i}", name=f"v{i}") for i in range(NT)]
    ptp = tc.alloc_tile_pool(name="ptp", bufs=2, side="right")

    xtp = tc.alloc_tile_pool(name="xtp", bufs=1, side="right")
    xt = [xtp.tile([P, T], BF16, tag=f"xt{j}", name=f"xt{j}") for j in range(ND)]
    wqk = tc.alloc_tile_pool(name="wqk", bufs=1, side="right")
    wq_sb = [wqk.tile([P, D], BF16, tag=f"wq{k}", name=f"wq{k}_sb")
             for k in range(ND)]
    wk_sb = [wqk.tile([P, D], BF16, tag=f"wk{k}", name=f"wk{k}_sb")
             for k in range(ND)]
    trA = tc.alloc_tile_pool(name="trA", bufs=1, side="right")
    wv_sb = [trA.tile([P, D], BF16, tag=f"wv{k}", name=f"wv{k}_sb")
             for k in range(ND)]

    # DMA order: first 4 x chunks, then wv, rest of x, then wq/wk.
    x_tiles = [None] * NT
    for i in range(4):
        x_t = trA.tile([P, D], F32, tag=f"xin{i % 6}", name="x_t")
        nc.sync.dma_start(x_t, io["x"][i * P:(i + 1) * P, :])
        x_tiles[i] = x_t
    for k in range(ND):
        nc.sync.dma_start(wv_sb[k], io["wv"][k * P:(k + 1) * P, :])
    for i in range(4, NT):
        x_t = trA.tile([P, D], F32, tag=f"xin{i % 6}", name="x_t")
        nc.sync.dma_start(x_t, io["x"][i * P:(i + 1) * P, :])
        x_tiles[i] = x_t
        if i == 9:
            for k in range(ND):
                nc.sync.dma_start(wq_sb[k], io["wq"][k * P:(k + 1) * P, :])
                nc.sync.dma_start(wk_sb[k], io["wk"][k * P:(k + 1) * P, :])
    nc.sync.dma_start(b2r, io["b2r"])
    nc.sync.dma_start(b1t, io["b1t"])

    # PE warm-up: dummy ident transposes keep the PE (and its HAM clock
    # gate) busy while the first x DMA + cast land.
    for w in range(8):
        ptw = psp.tile([P, P], BF16, tag="sm", bufs=2, name="ptw")
        nc.tensor.transpose(ptw, ident, ident)

    # ---- phase 0: cast + transpose (grouped chunks) + V matmul ----
    # first two groups are 2 chunks wide so PE work starts sooner
    groups = [(0, 2), (2, 2)] + [(s, 4) for s in range(4, NT, 4)]
    for gi, (g0, gw) in enumerate(groups):
        xbfs = []
        for ii in range(gw):
            i = g0 + ii
            x_bf = trA.tile([P, D], BF16, tag=f"xbf{i % 4}", name="x_bf")
            nc.scalar.copy(out=x_bf, in_=x_tiles[i])
            xbfs.append(x_bf)
        for j in range(ND):
            pt = psp.tile([P, SEQ], BF16, tag="sm", bufs=2, name="pt")
            for ii in range(gw):
                nc.tensor.transpose(pt[:, ii * P:(ii + 1) * P],
                                    xbfs[ii][:, j * P:(j + 1) * P], ident)
            dst = xt[j][:, g0 * P:(g0 + gw) * P]
            if (gi * ND + j) % 2 == 0:
                nc.vector.tensor_copy(out=dst, in_=pt[:, :gw * P])
            else:
                nc.scalar.copy(out=dst, in_=pt[:, :gw * P])
        for ii in range(gw):
            i = g0 + ii
            psv = psp.tile([P, 2 * SEQ], F32, tag="big", bufs=2, name="psv")
            for k in range(ND):
                lhs = xt[k][:, i * P:(i + 1) * P]
                nc.tensor.matmul(psv[:, :SEQ], lhs, wv_sb[k][:, :SEQ],
                                 start=(k == 0), stop=(k == ND - 1))
                nc.tensor.matmul(psv[:, SEQ:D], lhs, wv_sb[k][:, SEQ:D],
                                 start=(k == 0), stop=(k == ND - 1))
            nc.vector.memset(vN[i][:, HD::HD + 1], 1.0)
            if i % 2:
                nc.scalar.copy(
                    out=vN[i].rearrange("p (h w) -> p h w", w=HD + 1)[:, :, 0:HD],
                    in_=psv[:, :D].rearrange("p (h w) -> p h w", w=HD))
            else:
                nc.vector.tensor_copy(
                    out=vN[i].rearrange("p (h w) -> p h w", w=HD + 1)[:, :, 0:HD],
                    in_=psv[:, :D].rearrange("p (h w) -> p h w", w=HD))
    trA.release()
    dbg = globals().get("_DEBUG_DUMPS")
    if dbg:
        nc.sync.dma_start(dbg["d_xt0"], xt[0])
        nc.sync.dma_start(dbg["d_v0"], vN[0])

    # ---- left stack: attention output + persistent out1 (bf16) ----
    oTp = tc.alloc_tile_pool(name="oTp", bufs=1)
    oT = [oTp.tile([P, T], BF16, tag=f"oT{j}", name=f"oT{j}") for j in range(ND)]

    # ---- merged qkT + attention loop, per head pair m ----
    def qk_block(mm, g, which):
        """Project one 512-token column of qT[mm] (which=0) or kT[mm] (1)."""
        dst, w_sb = (qT[mm], wq_sb) if which == 0 else (kT[mm], wk_sb)
        pss = psp.tile([P, SEQ], F32, tag="sm", bufs=2, name="pss")
        for k in range(ND):
            nc.tensor.matmul(
                pss, w_sb[k][:, mm * P:(mm + 1) * P],
                xt[k][:, g * SEQ:(g + 1) * SEQ],
                start=(k == 0), stop=(k == ND - 1))
        nc.scalar.copy(out=dst[:, g * SEQ:(g + 1) * SEQ], in_=pss)

    for g in range(NB):
        qk_block(0, g, 0)
        qk_block(0, g, 1)
    for m in range(ND):
        if m == ND - 1:
            wqk.release()
            xtp.release()
        for b in range(NB):
            pts = [None] * 4  # (h0,A),(h0,B),(h1,A),(h1,B)
            q0 = qT[m][0:HD, b * SEQ:(b + 1) * SEQ]
            q1 = qT[m][HD:P, b * SEQ:(b + 1) * SEQ]

            def scores(half):
                sts = []
                for hloc in range(2):
                    st = psp.tile([P, 2 * SEQ], F32, tag="big", bufs=2,
                                  name="st")
                    sts.append(st)
                for t2 in range(2):
                    c = half * 2 + t2
                    kc = slice(b * SEQ + c * P, b * SEQ + (c + 1) * P)
                    nc.tensor.matmul(sts[0][:, t2 * SEQ:(t2 + 1) * SEQ],
                                     kT[m][0:HD, kc], q0, start=True, stop=True)
                    nc.tensor.matmul(sts[1][:, t2 * SEQ:(t2 + 1) * SEQ],
                                     kT[m][HD:P, kc], q1, start=True, stop=True)
                for hloc in range(2):
                    idx = half * 2 + hloc
                    if idx in FAST_TILES:
                        pi = ptp.tile([P, 2 * SEQ], I16, tag=f"ptf{idx}",
                                      bufs=2, name="ptsf")
                        nc.vector.tensor_scalar(out=pi, in0=sts[hloc],
                                                scalar1=FE_C1, scalar2=FE_C2,
                                                op0=OP.mult, op1=OP.add)
                        pts[idx] = pi.bitcast(BF16)
                    else:
                        pb = ptp.tile([P, 2 * SEQ], BF16, tag=f"pt{idx}",
                                      bufs=2, name="pts")
                        nc.scalar.activation(out=pb, in_=sts[hloc], func=AF.Exp)
                        pts[idx] = pb

            # software pipeline: next pair's projections fill the PE while
            # this pair's exp tiles drain on ACT/DVE.
            scores(0)
            if m + 1 < ND:
                qk_block(m + 1, b, 0)
            scores(1)
            if m + 1 < ND:
                qk_block(m + 1, b, 1)
            # attn @ V and normalize, per head
            for hloc in range(2):
                h = 2 * m + hloc
                ot = psp.tile([HD + 1, SEQ], F32, tag="ot", bufs=2, name="ot")
                for c in range(4):
                    vblk = vN[b * 4 + c][:, h * (HD + 1):(h + 1) * (HD + 1)]
                    nc.tensor.matmul(
                        ot, vblk,
                        pts[(c // 2) * 2 + hloc][:, (c % 2) * SEQ:(c % 2 + 1) * SEQ],
                        start=(c == 0), stop=(c == 3))
                rd0 = ptp.tile([1, SEQ], F32, tag="rd0", bufs=3, name="rd0")
                nc.scalar.copy(out=rd0, in_=ot[HD:HD + 1, :])
                rd = ptp.tile([1, SEQ], F32, tag="rd", bufs=3, name="rd")
                nc.vector.reciprocal_approx_fast(out=rd, in_=rd0)
                bcs = ptp.tile([HD, SEQ], F32, tag="bcs", bufs=3, name="bcs")
                nc.gpsimd.partition_broadcast(bcs, rd, channels=HD)
                nc.vector.tensor_mul(
                    out=oT[m][hloc * HD:(hloc + 1) * HD, b * SEQ:(b + 1) * SEQ],
                    in0=ot[0:HD, :], in1=bcs)
        if dbg and m == 0:
            nc.sync.dma_start(dbg["d_qT0"], qT[0])
            nc.sync.dma_start(dbg["d_kT0"], kT[0])
            nc.sync.dma_start(dbg["d_oT0"], oT[0])

    ptp.release()
    qkvp.release()

    # w1 loads, column-chunk first so the FFN's first hp matmuls start early
    w1p = tc.alloc_tile_pool(name="w1p", bufs=1, side="right")
    w1_sb = [w1p.tile([P, FF], BF16, tag=f"w1_{k}", name=f"w1_{k}")
             for k in range(ND)]
    for c in range(4):
        for k in range(ND):
            nc.sync.dma_start(w1_sb[k][:, c * D:(c + 1) * D],
                              io["w1"][k * P:(k + 1) * P, c * D:(c + 1) * D])

    # ---- Wo + LN1 phase (o1 transposes interleaved per 4-chunk group) ----
    o1bp = tc.alloc_tile_pool(name="o1bp", bufs=1)
    o1b = [o1bp.tile([P, D], BF16, tag=f"o1b{i}", name=f"o1b{i}")
           for i in range(NT)]
    o1Tp = tc.alloc_tile_pool(name="o1Tp", bufs=1)
    o1T = [o1Tp.tile([P, T], BF16, tag=f"o1T{j}", name=f"o1T{j}")
           for j in range(ND)]
    wop = tc.alloc_tile_pool(name="wop", bufs=1)
    wo_sb = []
    for k in range(ND):
        t = wop.tile([P, DA], BF16, tag=f"wo{k}", name=f"wo{k}_sb")
        nc.sync.dma_start(t, io["wo"][k * P:(k + 1) * P, :])
        wo_sb.append(t)
    trB = tc.alloc_tile_pool(name="trB", bufs=2)
    for i in range(NT):
        x_t = trB.tile([P, D], F32, tag="xin2", bufs=2, name="x_t2")
        nc.sync.dma_start(x_t, io["x"][i * P:(i + 1) * P, :])
        mh = psp.tile([P, 2 * SEQ], F32, tag="big", bufs=2, name="mh")
        for k in range(ND):
            lhs = oT[k][:, i * P:(i + 1) * P]
            nc.tensor.matmul(mh[:, :SEQ], lhs, wo_sb[k][:, :SEQ],
                             start=(k == 0), stop=(k == ND - 1))
            nc.tensor.matmul(mh[:, SEQ:DA], lhs, wo_sb[k][:, SEQ:DA],
                             start=(k == 0), stop=(k == ND - 1))
        y = trB.tile([P, D], BF16, tag="y1", bufs=2, name="y1")
        _ln_norm(nc, trB, mh, "1", eps_t, y)
        if ln_trivial:
            eng = nc.gpsimd if i % 4 == 0 else nc.vector
            eng.tensor_add(out=o1b[i], in0=x_t, in1=y)
        else:
            yw = trB.tile([P, D], BF16, tag="yw", bufs=2, name="yw")
            nc.vector.tensor_mul(out=yw, in0=y, in1=lnw["ln1w"])
            xb = trB.tile([P, D], F32, tag="xb", bufs=2, name="xb")
            nc.gpsimd.tensor_add(out=xb, in0=x_t, in1=lnw["ln1b"])
            nc.gpsimd.tensor_add(out=o1b[i], in0=xb, in1=yw)
        if i % 4 == 3:
            # o1 transposes for this 4-chunk group, copies spread ACT/DVE
            g = i // 4
            for j in range(ND):
                pt = psp.tile([P, SEQ], BF16, tag="sm", bufs=2, name="pt2")
                for ii in range(4):
                    nc.tensor.transpose(pt[:, ii * P:(ii + 1) * P],
                                        o1b[g * 4 + ii][:, j * P:(j + 1) * P],
                                        ident)
                if j % 2 == 0:
                    nc.vector.tensor_copy(
                        out=o1T[j][:, g * SEQ:(g + 1) * SEQ], in_=pt)
                else:
                    nc.scalar.copy(out=o1T[j][:, g * SEQ:(g + 1) * SEQ],
                                   in_=pt)
    trB.release()
    wop.release()

    # w2 loads (overlap with start of the FFN)
    w2p = tc.alloc_tile_pool(name="w2p", bufs=1, side="right")
    w2_sb = [w2p.tile([P, DA], BF16, tag=f"w2_{k}", name=f"w2_{k}")
             for k in range(NF)]
    for k in range(NF):
        nc.sync.dma_start(w2_sb[k], io["w2"][k * P:(k + 1) * P, :])

    # ---- FFN + LN2 + final residual ----
    trC = tc.alloc_tile_pool(name="trC", bufs=2)
    hbuf = tc.alloc_tile_pool(name="hbuf", bufs=1)
    NPRE = 3  # h chunks of group g+1 precomputed inside group g's sc loop

    def hp_gelu(g, f):
        hp = psp.tile([P, SEQ], F32, tag="sm", bufs=2, name="hp")
        for k in range(ND):
            nc.tensor.matmul(
                hp, w1_sb[k][:, f * P:(f + 1) * P],
                o1T[k][:, g * SEQ:(g + 1) * SEQ],
                start=(k == 0), stop=(k == ND - 1))
        ht = hbuf.tile([P, SEQ], BF16, tag=f"ht{f}",
                       bufs=2 if f < NPRE else 1, name=f"ht{f}")
        nc.scalar.activation(out=ht, in_=hp, func=AF.Gelu,
                             bias=b1t[:, f:f + 1], scale=1.0)
        return ht

    pre = {}
    for g in range(NB):
        hts = pre.pop(g, [])
        for f in range(len(hts), NF):
            hts.append(hp_gelu(g, f))
        for sc in range(4):
            i = g * 4 + sc
            fp = psp.tile([P, 2 * SEQ], F32, tag="big", bufs=2, name="fp")
            for f in range(NF):
                lhs = hts[f][:, sc * P:(sc + 1) * P]
                last = bias_zero and f == NF - 1
                nc.tensor.matmul(fp[:, :SEQ], lhs, w2_sb[f][:, :SEQ],
                                 start=(f == 0), stop=last)
                nc.tensor.matmul(fp[:, SEQ:DA], lhs, w2_sb[f][:, SEQ:DA],
                                 start=(f == 0), stop=last)
            if not bias_zero:
                nc.tensor.matmul(fp[:, :SEQ], ones1, b2r[:, :SEQ],
                                 start=False, stop=True)
                nc.tensor.matmul(fp[:, SEQ:DA], ones1, b2r[:, SEQ:DA],
                                 start=False, stop=True)
            if sc == 1 and g + 1 < NB:
                pre[g + 1] = [hp_gelu(g + 1, f) for f in range(NPRE)]
            y2 = trC.tile([P, D], BF16, tag="y2", bufs=2, name="y2")
            _ln_norm(nc, trC, fp, "2", eps_t, y2)
            outt = trC.tile([P, D], F32, tag="outt", bufs=3, name="outt")
            if ln_trivial:
                # keep the very tail off the slow gpsimd engine
                eng = nc.vector if (g == NB - 1 and sc >= 2) else nc.gpsimd
                eng.tensor_add(out=outt, in0=o1b[i], in1=y2)
            else:
                yw2 = trC.tile([P, D], BF16, tag="yw2", bufs=2, name="yw2")
                nc.vector.tensor_mul(out=yw2, in0=y2, in1=lnw["ln2w"])
                ob = trC.tile([P, D], F32, tag="ob", bufs=2, name="ob")
                nc.gpsimd.tensor_add(out=ob, in0=o1b[i], in1=lnw["ln2b"])
                nc.gpsimd.tensor_add(out=outt, in0=ob, in1=yw2)
            nc.sync.dma_start(io["out"][i * P:(i + 1) * P, :], outt)

    hbuf.release()
    trC.release()
    o1Tp.release()
    w2p.release()
    w1p.release()
    o1bp.release()
    oTp.release()
    consts.release()
    psp.release()


def build(bpc, ln_trivial, bias_zero):
    T = bpc * SEQ
    nc = bacc.Bacc("TRN2", target_bir_lowering=False, debug=False,
                   num_devices=N_CORES)
    io = {
        "x": nc.dram_tensor("x", [T, D], F32, kind="ExternalInput").ap(),
        "wq": nc.dram_tensor("wq", [D, D], BF16, kind="ExternalInput").ap(),
        "wk": nc.dram_tensor("wk", [D, D], BF16, kind="ExternalInput").ap(),
        "wv": nc.dram_tensor("wv", [D, D], BF16, kind="ExternalInput").ap(),
        "wo": nc.dram_tensor("wo", [D, DA], BF16, kind="ExternalInput").ap(),
        "w1": nc.dram_tensor("w1", [D, FF], BF16, kind="ExternalInput").ap(),
        "w2": nc.dram_tensor("w2", [FF, DA], BF16, kind="ExternalInput").ap(),
        "b1t": nc.dram_tensor("b1t", [P, NF], F32, kind="ExternalInput").ap(),
        "b2r": nc.dram_tensor("b2r", [1, DA], BF16, kind="ExternalInput").ap(),
        "ln1w": nc.dram_tensor("ln1w", [1, D], F32, kind="ExternalInput").ap(),
        "ln1b": nc.dram_tensor("ln1b", [1, D], F32, kind="ExternalInput").ap(),
        "ln2w": nc.dram_tensor("ln2w", [1, D], F32, kind="ExternalInput").ap(),
        "ln2b": nc.dram_tensor("ln2b", [1, D], F32, kind="ExternalInput").ap(),
        "out": nc.dram_tensor("out", [T, D], F32, kind="ExternalOutput").ap(),
    }
    with tile.TileContext(nc) as tc:
        emit(nc, tc, io, bpc, ln_trivial, bias_zero)
    nc.compile()
    return nc


def prep_weights(inputs):
    """Host-side weight layout prep (numpy only)."""
    bf = mybir.dt.np(BF16)
    f32 = np.float32
    wq = (np.asarray(inputs["Wq"], f32).transpose(1, 0, 2).reshape(D, D)
          / np.sqrt(HD)).astype(bf)
    wk = np.asarray(inputs["Wk"], f32).transpose(1, 0, 2).reshape(D, D).astype(bf)
    wv = np.asarray(inputs["Wv"], f32).transpose(1, 0, 2).reshape(D, D).astype(bf)
    wo = np.asarray(inputs["Wo"], f32)
    wo_a = np.concatenate([wo, wo.sum(axis=1, keepdims=True)], axis=1)
    w2 = np.asarray(inputs["W2"], f32)
    w2_a = np.concatenate([w2, w2.sum(axis=1, keepdims=True)], axis=1)
    b2 = np.asarray(inputs["b2"], f32).reshape(1, D)
    b2_a = np.concatenate([b2, b2.sum(axis=1, keepdims=True)], axis=1)
    return {
        "wq": np.ascontiguousarray(wq),
        "wk": np.ascontiguousarray(wk),
        "wv": np.ascontiguousarray(wv),
        "wo": np.ascontiguousarray(wo_a.astype(bf)),
        "w1": np.asarray(inputs["W1"], f32).astype(bf),
        "w2": np.ascontiguousarray(w2_a.astype(bf)),
        "b1t": np.ascontiguousarray(
            np.asarray(inputs["b1"], f32).reshape(NF, P).T),
        "b2r": b2_a.astype(bf),
        "ln1w": np.asarray(inputs["ln1_w"], f32).reshape(1, D),
        "ln1b": np.asarray(inputs["ln1_b"], f32).reshape(1, D),
        "ln2w": np.asarray(inputs["ln2_w"], f32).reshape(1, D),
        "ln2b": np.asarray(inputs["ln2_b"], f32).reshape(1, D),
    }


def _ln_is_trivial(inputs):
    f32 = np.float32
    return (np.all(np.asarray(inputs["ln1_w"], f32) == 1.0)
            and np.all(np.asarray(inputs["ln2_w"], f32) == 1.0)
            and np.all(np.asarray(inputs["ln1_b"], f32) == 0.0)
            and np.all(np.asarray(inputs["ln2_b"], f32) == 0.0))


def _b2_is_zero(inputs):
    return bool(np.all(np.asarray(inputs["b2"], np.float32) == 0.0))


def make_in_maps(inputs):
    bpc = B_TOTAL // N_CORES
    w = prep_weights(inputs)
    x = np.asarray(inputs["x"], np.float32)
    in_maps = []
    for c in range(N_CORES):
        shard = np.ascontiguousarray(
            x[c * bpc:(c + 1) * bpc].reshape(bpc * SEQ, D))
        in_maps.append({"x": shard, **w})
    return in_maps


_cache = {}


def kernel(**inputs) -> np.ndarray:
    bpc = B_TOTAL // N_CORES
    bz = _b2_is_zero(inputs)
    key = ("triv" if _ln_is_trivial(inputs) else "gen") + ("bz" if bz else "")
    if key not in _cache:
        _cache[key] = build(bpc, key.startswith("triv"), bz)
    _cache["nc"] = _cache[key]
    nc = _cache[key]
    in_maps = make_in_maps(inputs)
    res = run_bass_kernel_spmd(nc, in_maps, list(range(N_CORES)))
    out = np.concatenate(
        [res.results[c]["out"].reshape(bpc, SEQ, D) for c in range(N_CORES)],
        axis=0)
    return np.ascontiguousarray(out.astype(np.float32))


# revision 46
# speedup vs baseline: 1.1737x; 1.1737x over previous
"""Trainium2 Bass kernel for a 1-layer transformer encoder.

Sharding: data-parallel over batch -- each of the 8 cores gets 4 full
sequences (2048 tokens) and all weights; no collectives.

Structure (547us vs the 618us reproduced / 683us stated baseline):
- Per head-pair m: project qT/kT for pair m+1 software-pipelined between
  this pair's score halves and attnV, so the PE streams projections while
  softmax exp drains on ACT/DVE.
- Score matmuls for the two heads of a pair emitted adjacently at PE tile
  positions (0,0)/(64,0); the row-groups execute concurrently (~2x).
- 50% of exp tiles via a 1-op DVE bit-trick (Schraudolph in bf16 space:
  i16 = s*log2e*128 + 16249.6, bitcast to bf16), rest accurate on ACT.
- Softmax normalize: den row from the ones-column of V, ACT copy to SBUF,
  DVE reciprocal_approx_fast, gpsimd partition_broadcast, one fused
  psum*bcast multiply into oT (no DRAM bounce).
- LN: bn_stats/bn_aggr on DVE, sqrt on ACT, normalize via ACT Identity
  with per-partition scale/bias; gain/bias elided when inputs are the
  literal ones/zeros from setup_inputs (general path kept as fallback);
  b2 bias matmuls skipped when b2 == 0.
- out1 kept in SBUF as bf16 (no DRAM spill); o1 transposes + copies
  emitted inside the Wo loop so FFN starts without queue stalls; w1
  loaded column-major; transpose copies batched 4-per-instruction;
  PSUM: 2x[128,1024] + 2x[128,512] + 2x[65,512] tags = 8 banks.

Known hardware quirks worked around: reciprocal_approx_fast misreads
PSUM at partition base 64; partition bases must be 32-aligned; DVE ops
may read only one PSUM operand; ACT table reloads cost 1.28us.
"""

import os
import sys

import numpy as np

for _p in ("/opt/trn_rl_repo", "/root/.axon_site/_ro/trn_rl_repo"):
    if os.path.isdir(_p) and _p not in sys.path:
        sys.path.insert(0, _p)

os.environ.setdefault("MYCRO_LOCAL_CACHE", "1")

import concourse.bacc as bacc
import concourse.tile as tile
from concourse import mybir
from concourse.bass_utils import run_bass_kernel_spmd
from concourse.masks import make_identity

F32 = mybir.dt.float32
BF16 = mybir.dt.bfloat16
I16 = mybir.dt.int16
AF = mybir.ActivationFunctionType
OP = mybir.AluOpType

# model dims
D, NHEAD, HD, FF, SEQ, P = 768, 12, 64, 3072, 512, 128
ND = D // P    # 6 feature chunks
NF = FF // P   # 24 hidden chunks
EPS = 1e-5
N_CORES = 8
B_TOTAL = 32
DA = D + 1     # aug width (row-sum col for LN mean)

# Schraudolph fast-exp in bf16 bit space: bits_i16 = s*log2e*128 + C2
FE_C1 = float(1.4426950408889634 * 128.0)
FE_C2 = 16249.6

FAST_TILES = (1, 3)   # which of the 4 (half,head) exp tiles use DVE fastexp


def _ln_norm(nc, pool, ps, tag, eps_t, y_out):
    """y_out = layernorm(ps[:, :D]).

    Stats on DVE (bn_stats), sqrt on ACT, normalize on ACT (Identity with
    per-partition scale/bias) -- keeps both engines lightly loaded."""
    stats = pool.tile([P, 3, 6], F32, tag=f"bs{tag}", bufs=2, name=f"bs{tag}")
    for sg in range(3):
        nc.vector.bn_stats(out=stats[:, sg, :], in_=ps[:, sg * 256:(sg + 1) * 256])
    mv = pool.tile([P, 2], F32, tag=f"mv{tag}", bufs=2, name=f"mv{tag}")
    nc.vector.bn_aggr(out=mv, in_=stats)
    sd = pool.tile([P, 1], F32, tag=f"sd{tag}", bufs=2, name=f"sd{tag}")
    nc.scalar.activation(out=sd, in_=mv[:, 1:2], func=AF.Sqrt, bias=eps_t,
                         scale=1.0)
    rstd = pool.tile([P, 1], F32, tag=f"rstd{tag}", bufs=2, name=f"rstd{tag}")
    nc.vector.reciprocal_approx_fast(out=rstd, in_=sd)
    nm = pool.tile([P, 1], F32, tag=f"nm{tag}", bufs=2, name=f"nm{tag}")
    nc.vector.tensor_scalar(out=nm, in0=mv[:, 0:1], scalar1=rstd, scalar2=-1.0,
                            op0=OP.mult, op1=OP.mult)
    nc.scalar.activation(out=y_out, in_=ps[:, 0:D], func=AF.Identity,
                         bias=nm, scale=rstd)


def emit(nc, tc, io, bpc, ln_trivial, bias_zero):
    T = bpc * SEQ
    NT = T // P          # 16 token chunks
    NB = bpc             # 4 batches
    VW = NHEAD * (HD + 1)

    consts = tc.alloc_tile_pool(name="consts", bufs=1)
    psp = tc.alloc_tile_pool(name="psp", bufs=1, space="PSUM")

    # ---- constants ----
    ident = consts.tile([P, P], BF16)
    make_identity(nc, ident)
    ones1 = consts.tile([1, P], BF16)
    nc.vector.memset(ones1, 1.0)
    eps_t = consts.tile([P, 1], F32)
    nc.vector.memset(eps_t, EPS)
    b2r = consts.tile([1, DA], BF16)
    b1t = consts.tile([P, NF], F32)
    lnw = {}
    if not ln_trivial:
        for nm in ("ln1w", "ln1b", "ln2w", "ln2b"):
            t = consts.tile([P, D], F32, tag=nm, name=nm)
            nc.gpsimd.dma_start(t, io[nm].broadcast_to([P, D]))
            lnw[nm] = t

    # ---- right stack ----
    qkvp = tc.alloc_tile_pool(name="qkvp", bufs=1, side="right")
    qT = [qkvp.tile([P, T], BF16, tag=f"qT{j}", name=f"qT{j}") for j in range(ND)]
    kT = [qkvp.tile([P, T], BF16, tag=f"kT{j}", name=f"kT{j}") for j in range(ND)]
    vN = [qkvp.tile([P, VW], BF16, tag=f"v{i}", name=f"v{i}") for i in range(NT)]
    ptp = tc.alloc_tile_pool(name="ptp", bufs=2, side="right")

    xtp = tc.alloc_tile_pool(name="xtp", bufs=1, side="right")
    xt = [xtp.tile([P, T], BF16, tag=f"xt{j}", name=f"xt{j}") for j in range(ND)]
    wqk = tc.alloc_tile_pool(name="wqk", bufs=1, side="right")
    wq_sb = [wqk.tile([P, D], BF16, tag=f"wq{k}", name=f"wq{k}_sb")
             for k in range(ND)]
    wk_sb = [wqk.tile([P, D], BF16, tag=f"wk{k}", name=f"wk{k}_sb")
             for k in range(ND)]
    trA = tc.alloc_tile_pool(name="trA", bufs=1, side="right")
    wv_sb = [trA.tile([P, D], BF16, tag=f"wv{k}", name=f"wv{k}_sb")
             for k in range(ND)]

    # DMA order: first 4 x chunks, then wv, rest of x, then wq/wk.
    x_tiles = [None] * NT
    for i in range(4):
        x_t = trA.tile([P, D], F32, tag=f"xin{i % 6}", name="x_t")
        nc.sync.dma_start(x_t, io["x"][i * P:(i + 1) * P, :])
        x_tiles[i] = x_t
    for k in range(ND):
        nc.sync.dma_start(wv_sb[k], io["wv"][k * P:(k + 1) * P, :])
    for i in range(4, NT):
        x_t = trA.tile([P, D], F32, tag=f"xin{i % 6}", name="x_t")
        nc.sync.dma_start(x_t, io["x"][i * P:(i + 1) * P, :])
        x_tiles[i] = x_t
        if i == 9:
            for k in range(ND):
                nc.sync.dma_start(wq_sb[k], io["wq"][k * P:(k + 1) * P, :])
                nc.sync.dma_start(wk_sb[k], io["wk"][k * P:(k + 1) * P, :])
    nc.sync.dma_start(b2r, io["b2r"])
    nc.sync.dma_start(b1t, io["b1t"])

    # PE warm-up: dummy ident transposes keep the PE (and its HAM clock
    # gate) busy while the first x DMA + cast land.
    for w in range(8):
        ptw = psp.tile([P, P], BF16, tag="sm", bufs=2, name="ptw")
        nc.tensor.transpose(ptw, ident, ident)

    # ---- phase 0: cast + transpose (grouped chunks) + V matmul ----
    # first two groups are 2 chunks wide so PE work starts sooner
    groups = [(0, 2), (2, 2)] + [(s, 4) for s in range(4, NT, 4)]
    for gi, (g0, gw) in enumerate(groups):
        xbfs = []
        for ii in range(gw):
            i = g0 + ii
            x_bf = trA.tile([P, D], BF16, tag=f"xbf{i % 4}", name="x_bf")
            nc.scalar.copy(out=x_bf, in_=x_tiles[i])
            xbfs.append(x_bf)
        for j in range(ND):
            pt = psp.tile([P, SEQ], BF16, tag="sm", bufs=2, name="pt")
            for ii in range(gw):
                nc.tensor.transpose(pt[:, ii * P:(ii + 1) * P],
                                    xbfs[ii][:, j * P:(j + 1) * P], ident)
            dst = xt[j][:, g0 * P:(g0 + gw) * P]
            if (gi * ND + j) % 2 == 0:
                nc.vector.tensor_copy(out=dst, in_=pt[:, :gw * P])
            else:
                nc.scalar.copy(out=dst, in_=pt[:, :gw * P])
        for ii in range(gw):
            i = g0 + ii
            psv = psp.tile([P, 2 * SEQ], F32, tag="big", bufs=2, name="psv")
            for k in range(ND):
                lhs = xt[k][:, i * P:(i + 1) * P]
                nc.tensor.matmul(psv[:, :SEQ], lhs, wv_sb[k][:, :SEQ],
                                 start=(k == 0), stop=(k == ND - 1))
                nc.tensor.matmul(psv[:, SEQ:D], lhs, wv_sb[k][:, SEQ:D],
                                 start=(k == 0), stop=(k == ND - 1))
            nc.vector.memset(vN[i][:, HD::HD + 1], 1.0)
            if i % 2:
                nc.scalar.copy(
                    out=vN[i].rearrange("p (h w) -> p h w", w=HD + 1)[:, :, 0:HD],
                    in_=psv[:, :D].rearrange("p (h w) -> p h w", w=HD))
            else:
                nc.vector.tensor_copy(
                    out=vN[i].rearrange("p (h w) -> p h w", w=HD + 1)[:, :, 0:HD],
                    in_=psv[:, :D].rearrange("p (h w) -> p h w", w=HD))
    trA.release()
    dbg = globals().get("_DEBUG_DUMPS")
    if dbg:
        nc.sync.dma_start(dbg["d_xt0"], xt[0])
        nc.sync.dma_start(dbg["d_v0"], vN[0])

    # ---- left stack: attention output + persistent out1 (bf16) ----
    oTp = tc.alloc_tile_pool(name="oTp", bufs=1)
    oT = [oTp.tile([P, T], BF16, tag=f"oT{j}", name=f"oT{j}") for j in range(ND)]

    # ---- merged qkT + attention loop, per head pair m ----
    def qk_block(mm, g, which):
        """Project one 512-token column of qT[mm] (which=0) or kT[mm] (1)."""
        dst, w_sb = (qT[mm], wq_sb) if which == 0 else (kT[mm], wk_sb)
        pss = psp.tile([P, SEQ], F32, tag="sm", bufs=2, name="pss")
        for k in range(ND):
            nc.tensor.matmul(
                pss, w_sb[k][:, mm * P:(mm + 1) * P],
                xt[k][:, g * SEQ:(g + 1) * SEQ],
                start=(k == 0), stop=(k == ND - 1))
        nc.scalar.copy(out=dst[:, g * SEQ:(g + 1) * SEQ], in_=pss)

    for g in range(NB):
        qk_block(0, g, 0)
        qk_block(0, g, 1)
    for m in range(ND):
        if m == ND - 1:
            wqk.release()
            xtp.release()
        for b in range(NB):
            pts = [None] * 4  # (h0,A),(h0,B),(h1,A),(h1,B)
            q0 = qT[m][0:HD, b * SEQ:(b + 1) * SEQ]
            q1 = qT[m][HD:P, b * SEQ:(b + 1) * SEQ]

            def scores(half):
                sts = []
                for hloc in range(2):
                    st = psp.tile([P, 2 * SEQ], F32, tag="big", bufs=2,
                                  name="st")
                    sts.append(st)
                for t2 in range(2):
                    c = half * 2 + t2
                    kc = slice(b * SEQ + c * P, b * SEQ + (c + 1) * P)
                    nc.tensor.matmul(sts[0][:, t2 * SEQ:(t2 + 1) * SEQ],
                                     kT[m][0:HD, kc], q0, start=True, stop=True)
                    nc.tensor.matmul(sts[1][:, t2 * SEQ:(t2 + 1) * SEQ],
                                     kT[m][HD:P, kc], q1, start=True, stop=True)
                for hloc in range(2):
                    idx = half * 2 + hloc
                    if idx in FAST_TILES:
                        pi = ptp.tile([P, 2 * SEQ], I16, tag=f"ptf{idx}",
                                      bufs=2, name="ptsf")
                        nc.vector.tensor_scalar(out=pi, in0=sts[hloc],
                                                scalar1=FE_C1, scalar2=FE_C2,
                                                op0=OP.mult, op1=OP.add)
                        pts[idx] = pi.bitcast(BF16)
                    else:
                        pb = ptp.tile([P, 2 * SEQ], BF16, tag=f"pt{idx}",
                                      bufs=2, name="pts")
                        nc.scalar.activation(out=pb, in_=sts[hloc], func=AF.Exp)
                        pts[idx] = pb

            # software pipeline: next pair's projections fill the PE while
            # this pair's exp tiles drain on ACT/DVE.
            scores(0)
            if m + 1 < ND:
                qk_block(m + 1, b, 0)
            scores(1)
            if m + 1 < ND:
                qk_block(m + 1, b, 1)
            # attn @ V and normalize, per head
            for hloc in range(2):
                h = 2 * m + hloc
                ot = psp.tile([HD + 1, SEQ], F32, tag="ot", bufs=2, name="ot")
                for c in range(4):
                    vblk = vN[b * 4 + c][:, h * (HD + 1):(h + 1) * (HD + 1)]
                    nc.tensor.matmul(
                        ot, vblk,
                        pts[(c // 2) * 2 + hloc][:, (c % 2) * SEQ:(c % 2 + 1) * SEQ],
                        start=(c == 0), stop=(c == 3))
                rd0 = ptp.tile([1, SEQ], F32, tag="rd0", bufs=3, name="rd0")
                nc.scalar.copy(out=rd0, in_=ot[HD:HD + 1, :])
                rd = ptp.tile([1, SEQ], F32, tag="rd", bufs=3, name="rd")
                nc.vector.reciprocal_approx_fast(out=rd, in_=rd0)
                bcs = ptp.tile([HD, SEQ], F32, tag="bcs", bufs=3, name="bcs")
                nc.gpsimd.partition_broadcast(bcs, rd, channels=HD)
                nc.vector.tensor_mul(
                    out=oT[m][hloc * HD:(hloc + 1) * HD, b * SEQ:(b + 1) * SEQ],
                    in0=ot[0:HD, :], in1=bcs)
        if dbg and m == 0:
            nc.sync.dma_start(dbg["d_qT0"], qT[0])
            nc.sync.dma_start(dbg["d_kT0"], kT[0])
            nc.sync.dma_start(dbg["d_oT0"], oT[0])

    ptp.release()
    qkvp.release()

    # w1 loads, column-chunk first so the FFN's first hp matmuls start early
    w1p = tc.alloc_tile_pool(name="w1p", bufs=1, side="right")
    w1_sb = [w1p.tile([P, FF], BF16, tag=f"w1_{k}", name=f"w1_{k}")
             for k in range(ND)]
    for c in range(4):
        for k in range(ND):
            nc.sync.dma_start(w1_sb[k][:, c * D:(c + 1) * D],
                              io["w1"][k * P:(k + 1) * P, c * D:(c + 1) * D])

    # ---- Wo + LN1 phase (o1 transposes interleaved per 4-chunk group) ----
    o1bp = tc.alloc_tile_pool(name="o1bp", bufs=1)
    o1b = [o1bp.tile([P, D], BF16, tag=f"o1b{i}", name=f"o1b{i}")
           for i in range(NT)]
    o1Tp = tc.alloc_tile_pool(name="o1Tp", bufs=1)
    o1T = [o1Tp.tile([P, T], BF16, tag=f"o1T{j}", name=f"o1T{j}")
           for j in range(ND)]
    wop = tc.alloc_tile_pool(name="wop", bufs=1)
    wo_sb = []
    for k in range(ND):
        t = wop.tile([P, DA], BF16, tag=f"wo{k}", name=f"wo{k}_sb")
        nc.sync.dma_start(t, io["wo"][k * P:(k + 1) * P, :])
        wo_sb.append(t)
    trB = tc.alloc_tile_pool(name="trB", bufs=2)
    for i in range(NT):
        x_t = trB.tile([P, D], F32, tag="xin2", bufs=2, name="x_t2")
        nc.sync.dma_start(x_t, io["x"][i * P:(i + 1) * P, :])
        mh = psp.tile([P, 2 * SEQ], F32, tag="big", bufs=2, name="mh")
        for k in range(ND):
            lhs = oT[k][:, i * P:(i + 1) * P]
            nc.tensor.matmul(mh[:, :SEQ], lhs, wo_sb[k][:, :SEQ],
                             start=(k == 0), stop=(k == ND - 1))
            nc.tensor.matmul(mh[:, SEQ:DA], lhs, wo_sb[k][:, SEQ:DA],
                             start=(k == 0), stop=(k == ND - 1))
        y = trB.tile([P, D], BF16, tag="y1", bufs=2, name="y1")
        _ln_norm(nc, trB, mh, "1", eps_t, y)
        if ln_trivial:
            eng = nc.gpsimd if i % 2 == 0 else nc.vector
            eng.tensor_add(out=o1b[i], in0=x_t, in1=y)
        else:
            yw = trB.tile([P, D], BF16, tag="yw", bufs=2, name="yw")
            nc.vector.tensor_mul(out=yw, in0=y, in1=lnw["ln1w"])
            xb = trB.tile([P, D], F32, tag="xb", bufs=2, name="xb")
            nc.gpsimd.tensor_add(out=xb, in0=x_t, in1=lnw["ln1b"])
            nc.gpsimd.tensor_add(out=o1b[i], in0=xb, in1=yw)
        if i % 4 == 3:
            # o1 transposes for this 4-chunk group, copies spread ACT/DVE
            g = i // 4
            for j in range(ND):
                pt = psp.tile([P, SEQ], BF16, tag="sm", bufs=2, name="pt2")
                for ii in range(4):
                    nc.tensor.transpose(pt[:, ii * P:(ii + 1) * P],
                                        o1b[g * 4 + ii][:, j * P:(j + 1) * P],
                                        ident)
                if j % 2 == 0:
                    nc.vector.tensor_copy(
                        out=o1T[j][:, g * SEQ:(g + 1) * SEQ], in_=pt)
                else:
                    nc.scalar.copy(out=o1T[j][:, g * SEQ:(g + 1) * SEQ],
                                   in_=pt)
    trB.release()
    wop.release()

    # w2 loads (overlap with start of the FFN)
    w2p = tc.alloc_tile_pool(name="w2p", bufs=1, side="right")
    w2_sb = [w2p.tile([P, DA], BF16, tag=f"w2_{k}", name=f"w2_{k}")
             for k in range(NF)]
    for k in range(NF):
        nc.sync.dma_start(w2_sb[k], io["w2"][k * P:(k + 1) * P, :])

    # ---- FFN + LN2 + final residual ----
    trC = tc.alloc_tile_pool(name="trC", bufs=2)
    hbuf = tc.alloc_tile_pool(name="hbuf", bufs=1)
    for g in range(NB):
        hts = []
        for f in range(NF):
            hp = psp.tile([P, SEQ], F32, tag="sm", bufs=2, name="hp")
            for k in range(ND):
                nc.tensor.matmul(
                    hp, w1_sb[k][:, f * P:(f + 1) * P],
                    o1T[k][:, g * SEQ:(g + 1) * SEQ],
                    start=(k == 0), stop=(k == ND - 1))
            ht = hbuf.tile([P, SEQ], BF16, tag=f"ht{f}", name=f"ht{f}")
            nc.scalar.activation(out=ht, in_=hp, func=AF.Gelu,
                                 bias=b1t[:, f:f + 1], scale=1.0)
            hts.append(ht)
        for sc in range(4):
            i = g * 4 + sc
            fp = psp.tile([P, 2 * SEQ], F32, tag="big", bufs=2, name="fp")
            for f in range(NF):
                lhs = hts[f][:, sc * P:(sc + 1) * P]
                last = bias_zero and f == NF - 1
                nc.tensor.matmul(fp[:, :SEQ], lhs, w2_sb[f][:, :SEQ],
                                 start=(f == 0), stop=last)
                nc.tensor.matmul(fp[:, SEQ:DA], lhs, w2_sb[f][:, SEQ:DA],
                                 start=(f == 0), stop=last)
            if not bias_zero:
                nc.tensor.matmul(fp[:, :SEQ], ones1, b2r[:, :SEQ],
                                 start=False, stop=True)
                nc.tensor.matmul(fp[:, SEQ:DA], ones1, b2r[:, SEQ:DA],
                                 start=False, stop=True)
            y2 = trC.tile([P, D], BF16, tag="y2", bufs=2, name="y2")
            _ln_norm(nc, trC, fp, "2", eps_t, y2)
            outt = trC.tile([P, D], F32, tag="outt", bufs=3, name="outt")
            if ln_trivial:
                # keep the very tail off the slow gpsimd engine
                eng = nc.vector if (g == NB - 1 and sc >= 2) else nc.gpsimd
                eng.tensor_add(out=outt, in0=o1b[i], in1=y2)
            else:
                yw2 = trC.tile([P, D], BF16, tag="yw2", bufs=2, name="yw2")
                nc.vector.tensor_mul(out=yw2, in0=y2, in1=lnw["ln2w"])
                ob = trC.tile([P, D], F32, tag="ob", bufs=2, name="ob")
                nc.gpsimd.tensor_add(out=ob, in0=o1b[i], in1=lnw["ln2b"])
                nc.gpsimd.tensor_add(out=outt, in0=ob, in1=yw2)
            nc.sync.dma_start(io["out"][i * P:(i + 1) * P, :], outt)

    hbuf.release()
    trC.release()
    o1Tp.release()
    w2p.release()
    w1p.release()
    o1bp.release()
    oTp.release()
    consts.release()
    psp.release()


def build(bpc, ln_trivial, bias_zero):
    T = bpc * SEQ
    nc = bacc.Bacc("TRN2", target_bir_lowering=False, debug=False,
                   num_devices=N_CORES)
    io = {
        "x": nc.dram_tensor("x", [T, D], F32, kind="ExternalInput").ap(),
        "wq": nc.dram_tensor("wq", [D, D], BF16, kind="ExternalInput").ap(),
        "wk": nc.dram_tensor("wk", [D, D], BF16, kind="ExternalInput").ap(),
        "wv": nc.dram_tensor("wv", [D, D], BF16, kind="ExternalInput").ap(),
        "wo": nc.dram_tensor("wo", [D, DA], BF16, kind="ExternalInput").ap(),
        "w1": nc.dram_tensor("w1", [D, FF], BF16, kind="ExternalInput").ap(),
        "w2": nc.dram_tensor("w2", [FF, DA], BF16, kind="ExternalInput").ap(),
        "b1t": nc.dram_tensor("b1t", [P, NF], F32, kind="ExternalInput").ap(),
        "b2r": nc.dram_tensor("b2r", [1, DA], BF16, kind="ExternalInput").ap(),
        "ln1w": nc.dram_tensor("ln1w", [1, D], F32, kind="ExternalInput").ap(),
        "ln1b": nc.dram_tensor("ln1b", [1, D], F32, kind="ExternalInput").ap(),
        "ln2w": nc.dram_tensor("ln2w", [1, D], F32, kind="ExternalInput").ap(),
        "ln2b": nc.dram_tensor("ln2b", [1, D], F32, kind="ExternalInput").ap(),
        "out": nc.dram_tensor("out", [T, D], F32, kind="ExternalOutput").ap(),
    }
    with tile.TileContext(nc) as tc:
        emit(nc, tc, io, bpc, ln_trivial, bias_zero)
    nc.compile()
    return nc


def prep_weights(inputs):
    """Host-side weight layout prep (numpy only)."""
    bf = mybir.dt.np(BF16)
    f32 = np.float32
    wq = (np.asarray(inputs["Wq"], f32).transpose(1, 0, 2).reshape(D, D)
          / np.sqrt(HD)).astype(bf)
    wk = np.asarray(inputs["Wk"], f32).transpose(1, 0, 2).reshape(D, D).astype(bf)
    wv = np.asarray(inputs["Wv"], f32).transpose(1, 0, 2).reshape(D, D).astype(bf)
    wo = np.asarray(inputs["Wo"], f32)
    wo_a = np.concatenate([wo, wo.sum(axis=1, keepdims=True)], axis=1)
    w2 = np.asarray(inputs["W2"], f32)
    w2_a = np.concatenate([w2, w2.sum(axis=1, keepdims=True)], axis=1)
    b2 = np.asarray(inputs["b2"], f32).reshape(1, D)
    b2_a = np.concatenate([b2, b2.sum(axis=1, keepdims=True)], axis=1)
    return {
        "wq": np.ascontiguousarray(wq),
        "wk": np.ascontiguousarray(wk),
        "wv": np.ascontiguousarray(wv),
        "wo": np.ascontiguousarray(wo_a.astype(bf)),
        "w1": np.asarray(inputs["W1"], f32).astype(bf),
        "w2": np.ascontiguousarray(w2_a.astype(bf)),
        "b1t": np.ascontiguousarray(
            np.asarray(inputs["b1"], f32).reshape(NF, P).T),
        "b2r": b2_a.astype(bf),
        "ln1w": np.asarray(inputs["ln1_w"], f32).reshape(1, D),
        "ln1b": np.asarray(inputs["ln1_b"], f32).reshape(1, D),
        "ln2w": np.asarray(inputs["ln2_w"], f32).reshape(1, D),
        "ln2b": np.asarray(inputs["ln2_b"], f32).reshape(1, D),
    }


def _ln_is_trivial(inputs):
    f32 = np.float32
    return (np.all(np.asarray(inputs["ln1_w"], f32) == 1.0)
            and np.all(np.asarray(inputs["ln2_w"], f32) == 1.0)
            and np.all(np.asarray(inputs["ln1_b"], f32) == 0.0)
            and np.all(np.asarray(inputs["ln2_b"], f32) == 0.0))


def _b2_is_zero(inputs):
    return bool(np.all(np.asarray(inputs["b2"], np.float32) == 0.0))


def make_in_maps(inputs):
    bpc = B_TOTAL // N_CORES
    w = prep_weights(inputs)
    x = np.asarray(inputs["x"], np.float32)
    in_maps = []
    for c in range(N_CORES):
        shard = np.ascontiguousarray(
            x[c * bpc:(c + 1) * bpc].reshape(bpc * SEQ, D))
        in_maps.append({"x": shard, **w})
    return in_maps


_cache = {}


def kernel(**inputs) -> np.ndarray:
    bpc = B_TOTAL // N_CORES
    bz = _b2_is_zero(inputs)
    key = ("triv" if _ln_is_trivial(inputs) else "gen") + ("bz" if bz else "")
    if key not in _cache:
        _cache[key] = build(bpc, key.startswith("triv"), bz)
    _cache["nc"] = _cache[key]
    nc = _cache[key]
    in_maps = make_in_maps(inputs)
    res = run_bass_kernel_spmd(nc, in_maps, list(range(N_CORES)))
    out = np.concatenate(
        [res.results[c]["out"].reshape(bpc, SEQ, D) for c in range(N_CORES)],
        axis=0)
    return np.ascontiguousarray(out.astype(np.float32))
